# revision 2
# baseline (speedup 1.0000x reference)
"""Trainium2 Bass kernel for nn_MetricSelfAttention.

Math: the reference's softmax is dead code, so
    nudged = (p1 @ M @ p2^T) @ p1
reassociates to
    nudged = p1 @ (M @ (p2^T @ p1))        (per-head 64x64 Gram matrix G)
collapsing the O(W^2) attention matrices entirely.  The kernel is then
memory-bound: per core it reads x1[b]^T, x2[b] (8 MiB) and writes a partial
mixer product (4 MiB).

Sharding: 8 cores = 2 batches x 4 head-pairs.  Core (b, hg) computes heads
{2hg, 2hg+1} of batch b and the partial output
    out_partial = nudged[:, 128hg:128hg+128] @ W_mixer[:, 128hg:128hg+128].T
The host sums the 4 partials per batch and adds b_mixer.  The host also
passes x1 pre-transposed (pure layout prep, no FLOPs) so the kernel needs no
on-chip transposition of x1.

LayerNorm handling (all folded, no materialized normalized tensors):
  - gamma folds into the projection on the host; a nonzero beta enters as
    rank-1 bias matmuls (beta @ P'); omitted entirely when beta == 0.
  - x1 stats come from x1^T via PE ones-matmuls (sum and sum-of-squares
    rows), and LN enters the projection as
      p1^T = rstd_row * (P'^T @ x1^T + colsum(P') (x) (-mu_row))
    i.e. one extra rank-1 matmul into the projection PSUM plus a
    row-broadcast multiply on the PSUM->SBUF copy.
  - x2 is never normalized or transposed: with x2n = rstd2*x2 - mu2*rstd2
    rowwise,
      F := x2n^T @ p1  ==  x2^T @ (rstd2 * p1) - 1 (x) ((mu2*rstd2)^T @ p1)
    so F comes from raw x2 (natural layout, w-contraction, computed as F^T
    with a 512-wide moving operand) plus a rank-1 correction.

All large matmuls run in float32r (1 cycle/row at N>=256 vs 4 for fp32);
fp32r operands are rounded by their producers (DMA-cast on load, casting
PSUM->SBUF copies elsewhere).
"""

from contextlib import ExitStack

import numpy as np

import concourse.bacc as bacc
import concourse.bass as bass
import concourse.tile as tile
from concourse import mybir
from concourse.bass_utils import run_bass_kernel_spmd
from concourse.masks import make_identity

B, W, C, N, K = 2, 2048, 512, 8, 64
NCORES = 8
HPC = 2          # heads per core
K2 = HPC * K     # 128 channels per core
EPS = 1e-5
FP32 = mybir.dt.float32
FP32R = mybir.dt.float32r
AF = mybir.ActivationFunctionType
OP = mybir.AluOpType

NT = W // 128    # 16 w-tiles
NQ = W // 512    # 4 w-quads
NJ = C // 128    # 4 c-chunks


def _bcast_ap(row_ap, parts=128):
    """Partition-broadcast view of a [1, F] SBUF AP (step-0 partition dim),
    for DMA replication (same trick as groupnorm's bias broadcast)."""
    return bass.AP(tensor=row_ap.tensor, offset=row_ap.offset,
                   ap=[[0, parts]] + list(row_ap.ap[1:]))


def _body(ctx: ExitStack, tc: tile.TileContext, x1td, x2d, projd, mmatd,
          wmixd, colsumd, outd, pbiasrd, pbiascd):
    nc = tc.nc
    with_pbias = pbiasrd is not None

    persist = ctx.enter_context(tc.tile_pool(name="persist", bufs=1))
    sqpool = ctx.enter_context(tc.tile_pool(name="sq", bufs=3))
    rowpool = ctx.enter_context(tc.tile_pool(name="rows", bufs=2))
    bcpool = ctx.enter_context(tc.tile_pool(name="bc", bufs=2))
    spool = ctx.enter_context(tc.tile_pool(name="stats", bufs=8))
    outpool = ctx.enter_context(tc.tile_pool(name="outstage", bufs=2))
    ps_tp = ctx.enter_context(tc.tile_pool(name="ps_tp", bufs=2, space="PSUM"))
    ps_mm = ctx.enter_context(tc.tile_pool(name="ps_mm", bufs=2, space="PSUM"))
    ps_mo = ctx.enter_context(tc.tile_pool(name="ps_mo", bufs=2, space="PSUM"))

    # ---- constants / params -------------------------------------------------
    proj_s = persist.tile([128, NJ, K2], FP32)
    nc.sync.dma_start(out=proj_s, in_=projd.rearrange("(j p) k -> p j k", p=128))
    proj_r = persist.tile([128, NJ, K2], FP32R)
    nc.vector.tensor_copy(out=proj_r, in_=proj_s)
    wmix_s = persist.tile([K2, C], FP32)
    nc.sync.dma_start(out=wmix_s, in_=wmixd)
    wmix_r = persist.tile([K2, C], FP32R)
    nc.vector.tensor_copy(out=wmix_r, in_=wmix_s)
    mmat_s = persist.tile([K, HPC, K], FP32)
    nc.sync.dma_start(out=mmat_s, in_=mmatd)
    colsum_s = persist.tile([1, K2], FP32)
    nc.sync.dma_start(out=colsum_s, in_=colsumd)
    pbiasr_s = pbiasc_s = None
    if with_pbias:
        pbiasr_s = persist.tile([1, K2], FP32)
        nc.sync.dma_start(out=pbiasr_s, in_=pbiasrd)
        pbiasc_s = persist.tile([K2, 1], FP32)
        nc.sync.dma_start(out=pbiasc_s, in_=pbiascd)

    neg_ones512 = persist.tile([1, 512], FP32)
    nc.vector.memset(neg_ones512, -1.0)
    eps_s = persist.tile([128, 1], FP32)
    nc.vector.memset(eps_s, EPS)
    eps_row = persist.tile([1, 1], FP32)
    nc.vector.memset(eps_row, EPS)
    oneshalf = persist.tile([128, 1], FP32)
    nc.vector.memset(oneshalf, 1.0 / C)
    oneshalf_r = persist.tile([128, 1], FP32R)
    nc.vector.tensor_copy(out=oneshalf_r, in_=oneshalf)
    ident = persist.tile([128, 128], FP32)
    make_identity(nc, ident)
    ident_r = persist.tile([128, 128], FP32R)
    nc.vector.tensor_copy(out=ident_r, in_=ident)
    if with_pbias:
        ones_col = persist.tile([128, 1], FP32)
        nc.vector.memset(ones_col, 1.0)

    # ---- persistent activations --------------------------------------------
    x1t_r = persist.tile([128, NJ, W], FP32R)   # x1^T (rounded on DMA load)
    x2r_s = persist.tile([128, NT, C], FP32R)   # raw x2 (rounded on DMA load)
    rstd2_s = persist.tile([128, NT], FP32)     # per-row 1/std of x2
    pmr2_s = persist.tile([128, NT], FP32)      # per-row +mu*rstd of x2
    p1n_s = persist.tile([128, NT, K2], FP32)   # p1 natural
    p1s_s = persist.tile([128, NT, K2], FP32R)  # rstd2 * p1 (rowwise)
    p1T_s = persist.tile([K2, W], FP32R)        # p1^T
    ft_s = persist.tile([K2, C], FP32R)         # F^T
    f_s = persist.tile([128, NJ, K2], FP32)     # F (c on partitions)
    wrow_s = persist.tile([1, K2], FP32)        # (mu2*rstd2)^T @ p1
    g_s = persist.tile([K, HPC, K], FP32)       # per-head Gram
    h_bd_s = persist.tile([K2, K2], FP32R)      # block-diag H = M @ G
    nudgT_s = persist.tile([K2, W], FP32R)      # nudged^T
    if with_pbias:
        s1_s = persist.tile([1, K2], FP32)      # column sums of p1

    # ========================================================================
    # Front end, pipelined per w-quad: x1^T load -> stats rows -> projection;
    # x2 load -> row stats.
    # ========================================================================
    for q in range(NQ):
        qs = slice(q * 512, (q + 1) * 512)

        # x1^T quad load (fp32r DMA-cast)
        nc.gpsimd.dma_start(
            out=x1t_r[:, :, qs],
            in_=x1td[:, qs].rearrange("(j p) w -> p j w", p=128))

        # x2 quad load (fp32r DMA-cast) + bn stats
        nc.gpsimd.dma_start(
            out=x2r_s[:, 4 * q:4 * (q + 1), :],
            in_=x2d[qs, :].rearrange("(t p) c -> p t c", p=128))
        for t in range(4):
            tt = 4 * q + t
            stats = spool.tile([128, 6], FP32, tag="bst")
            nc.vector.bn_stats(stats, x2r_s[:, tt, :])
            mv = spool.tile([128, 2], FP32, tag="mv")
            nc.vector.bn_aggr(mv, stats)
            std = spool.tile([128, 1], FP32, tag="std")
            nc.scalar.activation(std, mv[:, 1:2], AF.Sqrt, bias=eps_s, scale=1.0)
            nc.vector.reciprocal(rstd2_s[:, tt:tt + 1], std)
            nc.vector.tensor_mul(pmr2_s[:, tt:tt + 1], mv[:, 0:1],
                                 rstd2_s[:, tt:tt + 1])

        # x1 stats rows via PE: mu_row = 1/C ones^T @ x1^T, ssq likewise
        mu_ps = ps_mo.tile([128, 512], FP32, tag="mo")
        ssq_ps = ps_mo.tile([128, 512], FP32, tag="mo")
        mu_psv = mu_ps[:1, :]
        ssq_psv = ssq_ps[:1, :]
        for j in range(NJ):
            sqt = sqpool.tile([128, 512], FP32R, tag="sq")
            nc.scalar.activation(sqt, x1t_r[:, j, qs], AF.Square)
            nc.tensor.matmul(mu_psv, lhsT=oneshalf_r, rhs=x1t_r[:, j, qs],
                             start=(j == 0), stop=(j == NJ - 1))
            nc.tensor.matmul(ssq_psv, lhsT=oneshalf_r, rhs=sqt,
                             start=(j == 0), stop=(j == NJ - 1))
        muq = rowpool.tile([1, 512], FP32, tag="muq")
        nc.scalar.copy(out=muq, in_=mu_psv)
        musq = rowpool.tile([1, 512], FP32, tag="musq")
        nc.vector.tensor_mul(musq, muq, muq)
        varq = rowpool.tile([1, 512], FP32, tag="varq")
        nc.vector.tensor_sub(varq, ssq_psv, musq)
        stdq = rowpool.tile([1, 512], FP32, tag="stdq")
        nc.scalar.activation(stdq, varq, AF.Sqrt, bias=eps_row, scale=1.0)
        rsq = rowpool.tile([1, 512], FP32, tag="rsq")
        nc.vector.reciprocal(rsq, stdq)
        nmq = rowpool.tile([1, 512], FP32, tag="nmq")
        nc.vector.tensor_scalar_mul(nmq, muq, -1.0)

        # broadcast rstd row across partitions
        bc1 = bcpool.tile([128, 512], FP32, tag="bc1")
        nc.gpsimd.partition_broadcast(bc1, rsq[0:1, :])

        # projection: p1^T = rstd_row * (P'^T @ x1^T + colsum (x) (-mu_row))
        pt = ps_mm.tile([128, 512], FP32, tag="mm")
        for j in range(NJ):
            nc.tensor.matmul(pt, lhsT=proj_r[:, j, :], rhs=x1t_r[:, j, qs],
                             start=(j == 0), stop=False)
        nc.tensor.matmul(pt, lhsT=colsum_s, rhs=nmq, start=False, stop=True)
        if with_pbias:
            tmp = bcpool.tile([128, 512], FP32, tag="ptmp")
            nc.vector.tensor_mul(tmp, pt, bc1)
            nc.vector.tensor_scalar_add(p1T_s[:, qs], tmp, pbiasc_s)
        else:
            nc.vector.tensor_mul(p1T_s[:, qs], pt, bc1)

        # p1 natural = PE-transpose of p1^T; p1s = rstd2-scaled rows
        for t in range(4):
            w_t = 4 * q + t
            ps = ps_tp.tile([128, 512], FP32R, tag="tp")
            nc.tensor.transpose(ps[:, :K2], p1T_s[:, w_t * 128:(w_t + 1) * 128],
                                ident_r)
            nc.scalar.copy(out=p1n_s[:, w_t, :], in_=ps[:, :K2])
            nc.vector.tensor_scalar_mul(p1s_s[:, w_t, :], p1n_s[:, w_t, :],
                                        rstd2_s[:, w_t:w_t + 1])

    # ========================================================================
    # Gram tail
    # ========================================================================
    # wrow = (mu2*rstd2)^T @ p1
    wp = ps_mm.tile([128, 512], FP32, tag="mm")
    wpv = wp[:1, :K2]
    for t in range(NT):
        nc.tensor.matmul(wpv, lhsT=pmr2_s[:, t:t + 1], rhs=p1n_s[:, t, :],
                         start=(t == 0), stop=(t == NT - 1))
    nc.vector.tensor_copy(out=wrow_s, in_=wpv)

    # F^T = p1s^T @ x2 - wrow (x) 1
    ftp = ps_mm.tile([128, 512], FP32, tag="mm")
    for t in range(NT):
        nc.tensor.matmul(ftp, lhsT=p1s_s[:, t, :], rhs=x2r_s[:, t, :],
                         start=(t == 0), stop=False)
    nc.tensor.matmul(ftp, lhsT=wrow_s, rhs=neg_ones512, start=False, stop=True)
    nc.vector.tensor_copy(out=ft_s, in_=ftp)

    # F = PE-transpose of F^T (c on partitions)
    fjp = ps_tp.tile([128, 4, 128], FP32R, tag="tp")
    for j in range(NJ):
        nc.tensor.transpose(fjp[:, j, :], ft_s[:, j * 128:(j + 1) * 128], ident_r)
    nc.scalar.copy(out=f_s, in_=fjp)

    # s1 = column sums of p1 (beta rank-1 term in G)
    if with_pbias:
        sp = ps_mm.tile([128, 512], FP32, tag="mm")
        spv = sp[:1, :K2]
        for t in range(NT):
            nc.tensor.matmul(spv, lhsT=ones_col, rhs=p1n_s[:, t, :],
                             start=(t == 0), stop=(t == NT - 1))
        nc.vector.tensor_copy(out=s1_s, in_=spv)

    # G_h = P'_h^T @ F_h (+ pbias_h (x) s1_h)
    gp = ps_mm.tile([128, 512], FP32, tag="mm")
    gpv = gp[:K, :HPC * K].rearrange("p (h k) -> p h k", h=HPC)
    for h in range(HPC):
        for j in range(NJ):
            nc.tensor.matmul(gpv[:, h, :],
                             lhsT=proj_s[:, j, h * K:(h + 1) * K],
                             rhs=f_s[:, j, h * K:(h + 1) * K],
                             start=(j == 0),
                             stop=(j == NJ - 1) and not with_pbias)
        if with_pbias:
            nc.tensor.matmul(gpv[:, h, :], lhsT=pbiasr_s[:, h * K:(h + 1) * K],
                             rhs=s1_s[:, h * K:(h + 1) * K],
                             start=False, stop=True)
    nc.vector.tensor_copy(out=g_s, in_=gpv)

    # H_h = M_h @ G_h  (M symmetric so lhsT = M_h); pack block-diagonal
    hp = ps_mm.tile([128, 512], FP32, tag="mm")
    hpv = hp[:, :K]
    for h in range(HPC):
        nc.tensor.matmul(hpv[h * K:(h + 1) * K, :], lhsT=mmat_s[:, h, :],
                         rhs=g_s[:, h, :])
    nc.vector.tensor_scalar_mul(h_bd_s, ident_r, 0.0)
    for h in range(HPC):
        nc.vector.tensor_copy(out=h_bd_s[h * K:(h + 1) * K, h * K:(h + 1) * K],
                              in_=hpv[h * K:(h + 1) * K, :])

    # nudged^T = H_bd^T @ p1^T;  mixer partial: out = nudged @ Wmix_slice
    for q in range(NQ):
        qs = slice(q * 512, (q + 1) * 512)
        ntp = ps_mm.tile([128, 512], FP32, tag="mm")
        nc.tensor.matmul(ntp, lhsT=h_bd_s, rhs=p1T_s[:, qs])
        nc.scalar.copy(out=nudgT_s[:, qs], in_=ntp)

        stage = outpool.tile([128, 4, C], FP32, tag="ostage")
        for t in range(4):
            w_t = q * 4 + t
            mo = ps_mo.tile([128, 512], FP32, tag="mo")
            nc.tensor.matmul(mo, lhsT=nudgT_s[:, w_t * 128:(w_t + 1) * 128],
                             rhs=wmix_r)
            if t % 2 == 0:
                nc.vector.tensor_copy(out=stage[:, t, :], in_=mo)
            else:
                nc.scalar.copy(out=stage[:, t, :], in_=mo)
        nc.sync.dma_start(
            out=outd[qs, :].rearrange("(t p) c -> p t c", p=128),
            in_=stage)


_PROGRAM_CACHE = {}


def _get_program(with_pbias: bool):
    key = ("nc", with_pbias)
    if key in _PROGRAM_CACHE:
        return _PROGRAM_CACHE[key]
    nc = bacc.Bacc("TRN2", debug=False, num_devices=NCORES)
    x1td = nc.dram_tensor("x1t", [C, W], FP32, kind="ExternalInput").ap()
    x2d = nc.dram_tensor("x2", [W, C], FP32, kind="ExternalInput").ap()
    projd = nc.dram_tensor("proj", [C, K2], FP32, kind="ExternalInput").ap()
    mmatd = nc.dram_tensor("mmat", [K, HPC, K], FP32, kind="ExternalInput").ap()
    wmixd = nc.dram_tensor("wmix", [K2, C], FP32, kind="ExternalInput").ap()
    colsumd = nc.dram_tensor("colsum", [1, K2], FP32, kind="ExternalInput").ap()
    pbiasrd = pbiascd = None
    if with_pbias:
        pbiasrd = nc.dram_tensor("pbiasr", [1, K2], FP32, kind="ExternalInput").ap()
        pbiascd = nc.dram_tensor("pbiasc", [K2, 1], FP32, kind="ExternalInput").ap()
    outd = nc.dram_tensor("out", [W, C], FP32, kind="ExternalOutput").ap()
    with tile.TileContext(nc) as tc:
        with ExitStack() as ctx:
            _body(ctx, tc, x1td, x2d, projd, mmatd, wmixd, colsumd, outd,
                  pbiasrd, pbiascd)
    nc.compile()
    _PROGRAM_CACHE[key] = nc
    return nc


def _host_prep(inputs):
    x1 = np.asarray(inputs["x1"], np.float32)
    x2 = np.ascontiguousarray(np.asarray(inputs["x2"], np.float32))
    gamma = np.asarray(inputs["gamma"], np.float32)
    beta = np.asarray(inputs["beta"], np.float32)
    proj = np.asarray(inputs["proj_nck"], np.float32)
    halves = np.asarray(inputs["halves"], np.float32)
    diagonals = np.asarray(inputs["diagonals"], np.float32)
    wmix = np.asarray(inputs["W_mixer"], np.float32)

    iu0, iu1 = np.triu_indices(K, k=1)
    m = np.zeros((N, K, K), np.float32)
    m[:, iu0, iu1] = halves
    m = m + np.swapaxes(m, -1, -2)
    d = np.arange(K)
    m[:, d, d] = diagonals

    pgam = proj * gamma[None, :, None]          # gamma folded into projection
    with_pbias = bool(np.any(beta))
    pbias = np.einsum("c,nck->nk", beta, pgam) if with_pbias else None

    x1t = [np.ascontiguousarray(x1[b].T) for b in range(B)]

    in_maps = []
    for core in range(NCORES):
        b, hg = divmod(core, NCORES // B)
        h0 = HPC * hg
        proj_core = np.ascontiguousarray(
            np.concatenate([pgam[h0 + i] for i in range(HPC)], axis=1))
        im = {
            "x1t": x1t[b],
            "x2": x2[b],
            "proj": proj_core,
            "mmat": np.ascontiguousarray(
                np.stack([m[h0 + i] for i in range(HPC)], axis=1)),
            "wmix": np.ascontiguousarray(
                wmix[:, K2 * hg:K2 * (hg + 1)].T),
            "colsum": np.ascontiguousarray(proj_core.sum(axis=0)[None, :]),
        }
        if with_pbias:
            pb = np.concatenate([pbias[h0 + i] for i in range(HPC)])
            im["pbiasr"] = np.ascontiguousarray(pb[None, :])
            im["pbiasc"] = np.ascontiguousarray(pb[:, None])
        in_maps.append(im)
    return in_maps, with_pbias


_TRACE = False
LAST_RESULT = None


def kernel(**inputs) -> np.ndarray:
    global LAST_RESULT
    in_maps, with_pbias = _host_prep(inputs)
    nc = _get_program(with_pbias)
    res = run_bass_kernel_spmd(nc, in_maps, core_ids=list(range(NCORES)),
                               trace=_TRACE)
    LAST_RESULT = res
    out = np.zeros((B, W, C), np.float32)
    for core in range(NCORES):
        b = core // (NCORES // B)
        out[b] += res.results[core]["out"]
    out += np.asarray(inputs["b_mixer"], np.float32)[None, None, :]
    return out



# revision 8
# speedup vs baseline: 1.2580x; 1.2580x over previous
"""Trainium2 Bass kernel for nn_MetricSelfAttention (v2, bf16).

Math: the reference's softmax is dead code, so
    nudged = (p1 @ M @ p2^T) @ p1
reassociates to
    nudged = p1 @ (M @ (p2^T @ p1))        (per-head 64x64 Gram matrix G)
collapsing the O(W^2) attention entirely.  The kernel is memory-bound.

Sharding: 8 cores = 2 batches x 4 head-pairs.  Core (b, hg) computes heads
{2hg, 2hg+1} of batch b and writes the partial mixer product
    out_partial = nudged[:, 128hg:+128] @ W_mixer[:, 128hg:+128].T
as bf16; the host sums the 4 partials per batch in fp32 and adds b_mixer.

All bulk tensors are bf16 (host-cast): halves HBM traffic vs fp32 and runs
the PE at 1 cycle/row irrespective of moving-dim size.  Per core the HBM
traffic is x1^T (2 MiB) + x2 (2 MiB) + out (2 MiB) + params (~0.3 MiB).

LayerNorm is computed on device and folded so no normalized tensor is ever
materialized:
  - x1 stats come from x1^T via PE ones-matmuls (mean and mean-square rows);
    the mean enters the projection as a rank-1 matmul (colsum (x) -mu), and
    1/std multiplies p1^T columns via one partition-broadcast row per quad
    (so the final output needs no row scaling at all: D1 rides inside p1T).
  - x2 is never normalized: with x2n = rstd2*(x2 - mu2) rowwise,
      F := x2n^T @ p1  ==  x2^T @ (rstd2*p1) - 1 (x) ((mu2*rstd2)^T @ p1)
    so F comes from raw x2 plus a rank-1 correction (v-row).
  - gamma folds into the projection on the host; nonzero beta enters as
    rank-1 bias matmuls (compiled only when beta != 0).

DMA: big transfers ride the two HWDGE queues (sync: x1t + stores,
scalar: x2 + stores), 512 KiB apiece; params go on gpsimd SWDGE.
"""

from contextlib import ExitStack

import numpy as np
import ml_dtypes

import concourse.bacc as bacc
import concourse.bass as bass
import concourse.tile as tile
from concourse import mybir
from concourse.bass_utils import run_bass_kernel_spmd
from concourse.masks import make_identity

B, W, C, N, K = 2, 2048, 512, 8, 64
NCORES = 8
HPC = 2          # heads per core
K2 = HPC * K     # 128 channels per core
EPS = 1e-5
FP32 = mybir.dt.float32
BF16 = mybir.dt.bfloat16
AF = mybir.ActivationFunctionType
OP = mybir.AluOpType
BF = ml_dtypes.bfloat16

NT = W // 128    # 16 w-tiles
NQ = W // 512    # 4 w-quads
NJ = C // 128    # 4 c-chunks


def _body(ctx: ExitStack, tc: tile.TileContext, x1td, x2d, pjd, colsumd, mmd,
          wmixTd, outd, pbrd, pbcd):
    nc = tc.nc
    with_pbias = pbrd is not None

    persist = ctx.enter_context(tc.tile_pool(name="persist", bufs=1))
    sqpool = ctx.enter_context(tc.tile_pool(name="sq", bufs=2))
    rowpool = ctx.enter_context(tc.tile_pool(name="rows", bufs=2))
    bcpool = ctx.enter_context(tc.tile_pool(name="bc", bufs=2))
    spool = ctx.enter_context(tc.tile_pool(name="stats", bufs=8))
    outpool = ctx.enter_context(tc.tile_pool(name="outstage", bufs=2))
    ps_st = ctx.enter_context(tc.tile_pool(name="ps_st", bufs=2, space="PSUM"))
    ps_mm = ctx.enter_context(tc.tile_pool(name="ps_mm", bufs=2, space="PSUM"))
    ps_tp = ctx.enter_context(tc.tile_pool(name="ps_tp", bufs=1, space="PSUM"))
    ps_f = ctx.enter_context(tc.tile_pool(name="ps_f", bufs=1, space="PSUM"))
    ps_mo = ctx.enter_context(tc.tile_pool(name="ps_mo", bufs=2, space="PSUM"))

    # ---- params (gpsimd SWDGE; small) --------------------------------------
    pj_s = persist.tile([128, NJ, K2], BF16)
    nc.gpsimd.dma_start(out=pj_s, in_=pjd.rearrange("(j p) k -> p j k", p=128))
    wmixT_s = persist.tile([K2, C], BF16)
    nc.gpsimd.dma_start(out=wmixT_s, in_=wmixTd)
    mm_s = persist.tile([K, HPC, K], BF16)
    nc.gpsimd.dma_start(out=mm_s, in_=mmd)
    colsum_s = persist.tile([1, K2], BF16)
    nc.gpsimd.dma_start(out=colsum_s, in_=colsumd)
    pbr_s = pbc_s = None
    if with_pbias:
        pbr_s = persist.tile([1, K2], BF16)
        nc.gpsimd.dma_start(out=pbr_s, in_=pbrd)
        pbc_s = persist.tile([K2, 1], FP32)
        nc.gpsimd.dma_start(out=pbc_s, in_=pbcd)
        ones_col = persist.tile([128, 1], BF16)
        nc.vector.memset(ones_col, 1.0)

    # ---- constants ---------------------------------------------------------
    neg_ones = persist.tile([1, 512], BF16)
    nc.vector.memset(neg_ones, -1.0)
    oneC = persist.tile([128, 1], BF16)
    nc.vector.memset(oneC, 1.0 / C)
    eps_s = persist.tile([128, 1], FP32)
    nc.vector.memset(eps_s, EPS)
    eps_row = persist.tile([1, 1], FP32)
    nc.vector.memset(eps_row, EPS)
    ident = persist.tile([128, 128], BF16)
    make_identity(nc, ident)

    # ---- persistent activations --------------------------------------------
    x1t_s = persist.tile([128, NJ, W], BF16)    # x1^T
    x2r_s = persist.tile([128, NT, C], BF16)    # raw x2
    p1T_s = persist.tile([K2, W], BF16)         # (D1 p1u)^T
    p1n_s = persist.tile([128, NT, K2], BF16)   # p1 natural
    p1s_s = persist.tile([128, NT, K2], BF16)   # rstd2 * p1
    rstd2_s = persist.tile([128, NT], FP32)
    m2r2_s = persist.tile([128, NT], BF16)      # mu2*rstd2 (bf16 lhsT)
    ft_s = persist.tile([K2, C], BF16)          # F^T
    f_s = persist.tile([128, NJ, K2], BF16)     # F natural
    g_s = persist.tile([K, HPC * K], BF16)      # Gram (2 heads side by side)
    hbd_s = persist.tile([K2, K2], BF16)        # block-diag H
    nudgT_s = persist.tile([K2, W], BF16)       # nudged^T
    vrow_s = persist.tile([1, K2], BF16)
    if with_pbias:
        s1_s = persist.tile([1, K2], BF16)

    nc.vector.memset(hbd_s, 0.0)

    # ========================================================================
    # Front end, pipelined per w-quad
    # ========================================================================
    for q in range(NQ):
        qs = slice(q * 512, (q + 1) * 512)

        nc.sync.dma_start(
            out=x1t_s[:, :, qs],
            in_=x1td[:, qs].rearrange("(j p) w -> p j w", p=128))
        nc.scalar.dma_start(
            out=x2r_s[:, 4 * q:4 * (q + 1), :],
            in_=x2d[qs, :].rearrange("(t p) c -> p t c", p=128))

        # x2 row stats per tile
        for t in range(4):
            tt = 4 * q + t
            st6 = spool.tile([128, 6], FP32, tag="bst")
            nc.vector.bn_stats(st6, x2r_s[:, tt, :])
            mv = spool.tile([128, 2], FP32, tag="mv")
            nc.vector.bn_aggr(mv, st6)
            std = spool.tile([128, 1], FP32, tag="std")
            nc.scalar.activation(std, mv[:, 1:2], AF.Sqrt, bias=eps_s, scale=1.0)
            nc.vector.reciprocal(rstd2_s[:, tt:tt + 1], std)
            nc.vector.tensor_mul(m2r2_s[:, tt:tt + 1], mv[:, 0:1],
                                 rstd2_s[:, tt:tt + 1])

        # x1 stats rows: mu = 1/C ones^T x1t ; ssq = 1/C ones^T (x1t^2)
        st_ps = ps_st.tile([33, 512], FP32, tag="st")
        mu_ps = st_ps[0:1, :]
        ssq_ps = st_ps[32:33, :]
        sq = sqpool.tile([128, NJ, 512], BF16, tag="sq")
        nc.gpsimd.tensor_mul(sq, x1t_s[:, :, qs], x1t_s[:, :, qs])
        for j in range(NJ):
            nc.tensor.matmul(mu_ps, lhsT=oneC, rhs=x1t_s[:, j, qs],
                             start=(j == 0), stop=(j == NJ - 1))
        for j in range(NJ):
            nc.tensor.matmul(ssq_ps, lhsT=oneC, rhs=sq[:, j, :],
                             start=(j == 0), stop=(j == NJ - 1))

        murow = rowpool.tile([1, 512], FP32, tag="murow")
        nc.scalar.copy(out=murow, in_=mu_ps)
        nmu = rowpool.tile([1, 512], BF16, tag="nmu")
        nc.vector.tensor_scalar_mul(nmu, murow, -1.0)
        musq = rowpool.tile([1, 512], FP32, tag="musq")
        nc.vector.tensor_mul(musq, murow, murow)
        varrow = rowpool.tile([1, 512], FP32, tag="var")
        nc.vector.tensor_sub(varrow, ssq_ps, musq)
        stdrow = rowpool.tile([1, 512], FP32, tag="stdr")
        nc.scalar.activation(stdrow, varrow, AF.Sqrt, bias=eps_row, scale=1.0)
        rstd1row = rowpool.tile([1, 512], FP32, tag="rs1")
        nc.vector.reciprocal(rstd1row, stdrow)
        bc1 = bcpool.tile([128, 512], FP32, tag="bc1")
        nc.gpsimd.partition_broadcast(bc1, rstd1row)

        # projection: p1u^T = P^T x1t + colsum^T (x) (-mu)
        pt = ps_mm.tile([128, 512], FP32, tag="mm")
        for j in range(NJ):
            nc.tensor.matmul(pt, lhsT=pj_s[:, j, :], rhs=x1t_s[:, j, qs],
                             start=(j == 0), stop=False)
        nc.tensor.matmul(pt, lhsT=colsum_s, rhs=nmu, start=False, stop=True)

        # p1^T = rstd1-row * p1u^T  (+ pbias column)
        if with_pbias:
            tmp = bcpool.tile([128, 512], FP32, tag="ptmp")
            nc.vector.tensor_mul(tmp, pt, bc1)
            nc.vector.tensor_scalar_add(p1T_s[:, qs], tmp, pbc_s)
        else:
            nc.vector.tensor_mul(p1T_s[:, qs], pt, bc1)

        # p1 natural (PE transpose) and p1s = rstd2 * p1
        tp = ps_tp.tile([128, 512], BF16, tag="tp")
        for t in range(4):
            w_t = 4 * q + t
            nc.tensor.transpose(tp[:, t * 128:(t + 1) * 128],
                                p1T_s[:, w_t * 128:(w_t + 1) * 128], ident)
        nc.scalar.copy(out=p1n_s[:, 4 * q:4 * (q + 1), :], in_=tp)
        for t in range(4):
            tt = 4 * q + t
            nc.vector.tensor_scalar_mul(p1s_s[:, tt, :], p1n_s[:, tt, :],
                                        rstd2_s[:, tt:tt + 1])

    # ========================================================================
    # Gram tail
    # ========================================================================
    # F^T = p1s^T @ x2 - v (x) 1,   v = (mu2*rstd2)^T @ p1n
    vtile = ps_st.tile([33, 512], FP32, tag="st")
    vps = vtile[0:1, :K2]
    for t in range(NT):
        nc.tensor.matmul(vps, lhsT=m2r2_s[:, t:t + 1], rhs=p1n_s[:, t, :],
                         start=(t == 0), stop=(t == NT - 1))
    nc.scalar.copy(out=vrow_s, in_=vps)

    ftp = ps_f.tile([K2, C], FP32, tag="f")
    for t in range(NT):
        nc.tensor.matmul(ftp, lhsT=p1s_s[:, t, :], rhs=x2r_s[:, t, :],
                         start=(t == 0), stop=False)
    nc.tensor.matmul(ftp, lhsT=vrow_s, rhs=neg_ones, start=False, stop=True)
    nc.scalar.copy(out=ft_s, in_=ftp)

    # F natural
    ftp2 = ps_tp.tile([128, 512], BF16, tag="tp")
    for j in range(NJ):
        nc.tensor.transpose(ftp2[:, j * 128:(j + 1) * 128],
                            ft_s[:, j * 128:(j + 1) * 128], ident)
    nc.scalar.copy(out=f_s, in_=ftp2)

    if with_pbias:
        sptile = ps_st.tile([33, 512], FP32, tag="st")
        sp = sptile[0:1, :K2]
        for t in range(NT):
            nc.tensor.matmul(sp, lhsT=ones_col, rhs=p1n_s[:, t, :],
                             start=(t == 0), stop=(t == NT - 1))
        nc.scalar.copy(out=s1_s, in_=sp)

    # G_h = P_h^T @ F_h (+ pb_h (x) s1_h)
    gp = ps_mm.tile([128, 512], FP32, tag="mm")
    gv = gp[:K, :HPC * K]
    for h in range(HPC):
        hs = slice(h * K, (h + 1) * K)
        for j in range(NJ):
            nc.tensor.matmul(gv[:, hs], lhsT=pj_s[:, j, hs], rhs=f_s[:, j, hs],
                             start=(j == 0),
                             stop=(j == NJ - 1) and not with_pbias)
        if with_pbias:
            nc.tensor.matmul(gv[:, hs], lhsT=pbr_s[:, hs], rhs=s1_s[:, hs],
                             start=False, stop=True)
    nc.scalar.copy(out=g_s, in_=gv)

    # H_h = M_h @ G_h; assemble block-diagonal (hbd zeroed at start)
    hp = ps_mm.tile([128, 512], FP32, tag="mm")
    hv = hp[:K, :HPC * K]
    for h in range(HPC):
        hs = slice(h * K, (h + 1) * K)
        nc.tensor.matmul(hv[:, hs], lhsT=mm_s[:, h, :], rhs=g_s[:, hs])
    for h in range(HPC):
        hs = slice(h * K, (h + 1) * K)
        nc.vector.tensor_copy(out=hbd_s[hs, hs], in_=hv[:K, hs])

    # nudged^T = H_bd^T @ p1^T;  out = nudged @ wmixT
    for q in range(NQ):
        qs = slice(q * 512, (q + 1) * 512)
        npp = ps_mm.tile([128, 512], FP32, tag="mm")
        nc.tensor.matmul(npp, lhsT=hbd_s, rhs=p1T_s[:, qs])
        nc.scalar.copy(out=nudgT_s[:, qs], in_=npp)

        stage = outpool.tile([128, 4, C], BF16, tag="ostage")
        for t in range(4):
            w_t = q * 4 + t
            mo = ps_mo.tile([128, 512], FP32, tag="mo")
            nc.tensor.matmul(mo, lhsT=nudgT_s[:, w_t * 128:(w_t + 1) * 128],
                             rhs=wmixT_s)
            if t % 2 == 0:
                nc.vector.tensor_copy(out=stage[:, t, :], in_=mo)
            else:
                nc.scalar.copy(out=stage[:, t, :], in_=mo)
        eng = nc.sync if q % 2 == 0 else nc.scalar
        eng.dma_start(
            out=outd[qs, :].rearrange("(t p) c -> p t c", p=128),
            in_=stage)


_PROGRAM_CACHE = {}


def _get_program(with_pbias: bool):
    key = ("v2", with_pbias)
    if key in _PROGRAM_CACHE:
        return _PROGRAM_CACHE[key]
    nc = bacc.Bacc("TRN2", debug=False, num_devices=NCORES)
    x1td = nc.dram_tensor("x1t", [C, W], BF16, kind="ExternalInput").ap()
    x2d = nc.dram_tensor("x2", [W, C], BF16, kind="ExternalInput").ap()
    pjd = nc.dram_tensor("pj", [C, K2], BF16, kind="ExternalInput").ap()
    colsumd = nc.dram_tensor("colsum", [1, K2], BF16, kind="ExternalInput").ap()
    mmd = nc.dram_tensor("mm", [K, HPC, K], BF16, kind="ExternalInput").ap()
    wmixTd = nc.dram_tensor("wmixT", [K2, C], BF16, kind="ExternalInput").ap()
    pbrd = pbcd = None
    if with_pbias:
        pbrd = nc.dram_tensor("pbr", [1, K2], BF16, kind="ExternalInput").ap()
        pbcd = nc.dram_tensor("pbc", [K2, 1], FP32, kind="ExternalInput").ap()
    outd = nc.dram_tensor("out", [W, C], BF16, kind="ExternalOutput").ap()
    with tile.TileContext(nc) as tc:
        with ExitStack() as ctx:
            _body(ctx, tc, x1td, x2d, pjd, colsumd, mmd, wmixTd, outd,
                  pbrd, pbcd)
    nc.compile()
    _PROGRAM_CACHE[key] = nc
    return nc


def _host_prep(inputs):
    x1 = np.asarray(inputs["x1"], np.float32)
    x2 = np.asarray(inputs["x2"], np.float32)
    gamma = np.asarray(inputs["gamma"], np.float32)
    beta = np.asarray(inputs["beta"], np.float32)
    proj = np.asarray(inputs["proj_nck"], np.float32)
    halves = np.asarray(inputs["halves"], np.float32)
    diagonals = np.asarray(inputs["diagonals"], np.float32)
    wmix = np.asarray(inputs["W_mixer"], np.float32)

    iu0, iu1 = np.triu_indices(K, k=1)
    m = np.zeros((N, K, K), np.float32)
    m[:, iu0, iu1] = halves
    m = m + np.swapaxes(m, -1, -2)
    d = np.arange(K)
    m[:, d, d] = diagonals

    pgam = proj * gamma[None, :, None]
    with_pbias = bool(np.any(beta))
    pbias = np.einsum("c,nck->nk", beta, pgam) if with_pbias else None

    x1t = [np.ascontiguousarray(x1[b].T.astype(BF)) for b in range(B)]
    x2b = [np.ascontiguousarray(x2[b].astype(BF)) for b in range(B)]

    in_maps = []
    for core in range(NCORES):
        b, hg = divmod(core, NCORES // B)
        h0 = HPC * hg
        pj_core = np.concatenate([pgam[h0 + i] for i in range(HPC)], axis=1)
        im = {
            "x1t": x1t[b],
            "x2": x2b[b],
            "pj": np.ascontiguousarray(pj_core.astype(BF)),
            "colsum": np.ascontiguousarray(
                pj_core.sum(axis=0)[None, :].astype(BF)),
            "mm": np.ascontiguousarray(
                np.stack([m[h0 + i] for i in range(HPC)], axis=1).astype(BF)),
            "wmixT": np.ascontiguousarray(
                wmix[:, K2 * hg:K2 * (hg + 1)].T.astype(BF)),
        }
        if with_pbias:
            pb = np.concatenate([pbias[h0 + i] for i in range(HPC)])
            im["pbr"] = np.ascontiguousarray(pb[None, :].astype(BF))
            im["pbc"] = np.ascontiguousarray(pb[:, None].astype(np.float32))
        in_maps.append(im)
    return in_maps, with_pbias


_TRACE = False
LAST_RESULT = None


def kernel(**inputs) -> np.ndarray:
    global LAST_RESULT
    in_maps, with_pbias = _host_prep(inputs)
    nc = _get_program(with_pbias)
    res = run_bass_kernel_spmd(nc, in_maps, core_ids=list(range(NCORES)),
                               trace=_TRACE)
    LAST_RESULT = res
    out = np.zeros((B, W, C), np.float32)
    for core in range(NCORES):
        b = core // (NCORES // B)
        out[b] += res.results[core]["out"].astype(np.float32)
    out += np.asarray(inputs["b_mixer"], np.float32)[None, None, :]
    return out


# revision 10
# speedup vs baseline: 1.3252x; 1.0534x over previous
"""Trainium2 Bass kernel for nn_MetricSelfAttention (v2, bf16).

Math: the reference's softmax is dead code, so
    nudged = (p1 @ M @ p2^T) @ p1
reassociates to
    nudged = p1 @ (M @ (p2^T @ p1))        (per-head 64x64 Gram matrix G)
collapsing the O(W^2) attention entirely.  The kernel is memory-bound.

Sharding: 8 cores = 2 batches x 4 head-pairs.  Core (b, hg) computes heads
{2hg, 2hg+1} of batch b and writes the partial mixer product
    out_partial = nudged[:, 128hg:+128] @ W_mixer[:, 128hg:+128].T
as bf16; the host sums the 4 partials per batch in fp32 and adds b_mixer.

All bulk tensors are bf16 (host-cast): halves HBM traffic vs fp32 and runs
the PE at 1 cycle/row irrespective of moving-dim size.  Per core the HBM
traffic is x1^T (2 MiB) + x2 (2 MiB) + out (2 MiB) + params (~0.3 MiB).

LayerNorm is computed on device and folded so no normalized tensor is ever
materialized:
  - x1 stats come from x1^T via PE ones-matmuls (mean and mean-square rows);
    the mean enters the projection as a rank-1 matmul (colsum (x) -mu), and
    1/std multiplies p1^T columns via one partition-broadcast row per quad
    (so the final output needs no row scaling at all: D1 rides inside p1T).
  - x2 is never normalized: with x2n = rstd2*(x2 - mu2) rowwise,
      F := x2n^T @ p1  ==  x2^T @ (rstd2*p1) - 1 (x) ((mu2*rstd2)^T @ p1)
    so F comes from raw x2 plus a rank-1 correction (v-row).
  - gamma folds into the projection on the host; nonzero beta enters as
    rank-1 bias matmuls (compiled only when beta != 0).

DMA: big transfers ride the two HWDGE queues (sync: x1t + stores,
scalar: x2 + stores), 512 KiB apiece; params go on gpsimd SWDGE.
"""

from contextlib import ExitStack

import numpy as np
import ml_dtypes

import concourse.bacc as bacc
import concourse.bass as bass
import concourse.tile as tile
from concourse import mybir
from concourse.bass_utils import run_bass_kernel_spmd
from concourse.masks import make_identity

B, W, C, N, K = 2, 2048, 512, 8, 64
NCORES = 8
HPC = 2          # heads per core
K2 = HPC * K     # 128 channels per core
EPS = 1e-5
FP32 = mybir.dt.float32
BF16 = mybir.dt.bfloat16
AF = mybir.ActivationFunctionType
OP = mybir.AluOpType
BF = ml_dtypes.bfloat16

NT = W // 128    # 16 w-tiles
NQ = W // 512    # 4 w-quads
NJ = C // 128    # 4 c-chunks


def _body(ctx: ExitStack, tc: tile.TileContext, x1td, x2d, pjd, colsumd, mmd,
          wmixTd, outd, pbrd, pbcd):
    nc = tc.nc
    with_pbias = pbrd is not None

    persist = ctx.enter_context(tc.tile_pool(name="persist", bufs=1))
    sqpool = ctx.enter_context(tc.tile_pool(name="sq", bufs=2))
    rowpool = ctx.enter_context(tc.tile_pool(name="rows", bufs=2))
    bcpool = ctx.enter_context(tc.tile_pool(name="bc", bufs=2))
    spool = ctx.enter_context(tc.tile_pool(name="stats", bufs=8))
    outpool = ctx.enter_context(tc.tile_pool(name="outstage", bufs=2))
    ps_st = ctx.enter_context(tc.tile_pool(name="ps_st", bufs=2, space="PSUM"))
    ps_mm = ctx.enter_context(tc.tile_pool(name="ps_mm", bufs=2, space="PSUM"))
    ps_tp = ctx.enter_context(tc.tile_pool(name="ps_tp", bufs=1, space="PSUM"))
    ps_f = ctx.enter_context(tc.tile_pool(name="ps_f", bufs=1, space="PSUM"))
    ps_mo = ctx.enter_context(tc.tile_pool(name="ps_mo", bufs=2, space="PSUM"))

    # ---- params (sync HWDGE; must land before the first matmul) ------------
    pj_s = persist.tile([128, NJ, K2], BF16)
    nc.sync.dma_start(out=pj_s, in_=pjd.rearrange("(j p) k -> p j k", p=128))
    colsum_s = persist.tile([1, K2], BF16)
    nc.sync.dma_start(out=colsum_s, in_=colsumd)
    wmixT_s = persist.tile([K2, C], BF16)
    nc.scalar.dma_start(out=wmixT_s, in_=wmixTd)
    mm_s = persist.tile([K, HPC, K], BF16)
    nc.scalar.dma_start(out=mm_s, in_=mmd)
    pbr_s = pbc_s = None
    if with_pbias:
        pbr_s = persist.tile([1, K2], BF16)
        nc.sync.dma_start(out=pbr_s, in_=pbrd)
        pbc_s = persist.tile([K2, 1], FP32)
        nc.sync.dma_start(out=pbc_s, in_=pbcd)
        ones_col = persist.tile([128, 1], BF16)
        nc.vector.memset(ones_col, 1.0)

    # ---- constants ---------------------------------------------------------
    neg_ones = persist.tile([1, 512], BF16)
    nc.vector.memset(neg_ones, -1.0)
    oneC = persist.tile([128, 1], BF16)
    nc.vector.memset(oneC, 1.0 / C)
    eps_s = persist.tile([128, 1], FP32)
    nc.vector.memset(eps_s, EPS)
    eps_row = persist.tile([1, 1], FP32)
    nc.vector.memset(eps_row, EPS)
    ident = persist.tile([128, 128], BF16)
    make_identity(nc, ident)

    # ---- persistent activations --------------------------------------------
    x1t_s = persist.tile([128, NJ, W], BF16)    # x1^T
    x2r_s = persist.tile([128, NT, C], BF16)    # raw x2
    p1T_s = persist.tile([K2, W], BF16)         # (D1 p1u)^T
    p1n_s = persist.tile([128, NT, K2], BF16)   # p1 natural
    p1s_s = persist.tile([128, NT, K2], BF16)   # rstd2 * p1
    rstd2_s = persist.tile([128, NT], FP32)
    m2r2_s = persist.tile([128, NT], BF16)      # mu2*rstd2 (bf16 lhsT)
    ft_s = persist.tile([K2, C], BF16)          # F^T
    f_s = persist.tile([128, NJ, K2], BF16)     # F natural
    g_s = persist.tile([K, HPC * K], BF16)      # Gram (2 heads side by side)
    hbd_s = persist.tile([K2, K2], BF16)        # block-diag H
    nudgT_s = persist.tile([K2, W], BF16)       # nudged^T
    vrow_s = persist.tile([1, K2], BF16)
    if with_pbias:
        s1_s = persist.tile([1, K2], BF16)

    nc.vector.memset(hbd_s, 0.0)

    # ========================================================================
    # Front end, pipelined per w-quad
    # ========================================================================
    ftp = ps_f.tile([K2, C], FP32, tag="f")
    for q in range(NQ):
        qs = slice(q * 512, (q + 1) * 512)

        nc.sync.dma_start(
            out=x1t_s[:, :, qs],
            in_=x1td[:, qs].rearrange("(j p) w -> p j w", p=128))
        nc.scalar.dma_start(
            out=x2r_s[:, 4 * q:4 * (q + 1), :],
            in_=x2d[qs, :].rearrange("(t p) c -> p t c", p=128))

        # x2 row stats: one bn_stats per quad, batched tail math
        st6 = spool.tile([128, 4, 6], FP32, tag="bst")
        mvq = spool.tile([128, 4, 2], FP32, tag="mv")
        for t in range(4):
            nc.vector.bn_stats(st6[:, t, :], x2r_s[:, 4 * q + t, :])
            nc.vector.bn_aggr(mvq[:, t, :], st6[:, t, :])
        stdq = spool.tile([128, 4], FP32, tag="std")
        nc.scalar.activation(stdq, mvq[:, :, 1], AF.Sqrt, bias=eps_s, scale=1.0)
        nc.vector.reciprocal(rstd2_s[:, 4 * q:4 * (q + 1)], stdq)
        nc.vector.tensor_mul(m2r2_s[:, 4 * q:4 * (q + 1)], mvq[:, :, 0],
                             rstd2_s[:, 4 * q:4 * (q + 1)])

        # x1 stats rows: mu = 1/C ones^T x1t ; ssq = 1/C ones^T (x1t^2)
        st_ps = ps_st.tile([33, 512], FP32, tag="st")
        mu_ps = st_ps[0:1, :]
        ssq_ps = st_ps[32:33, :]
        sq = sqpool.tile([128, NJ, 512], BF16, tag="sq")
        nc.vector.tensor_mul(sq[:, 0:2, :], x1t_s[:, 0:2, qs], x1t_s[:, 0:2, qs])
        nc.scalar.activation(sq[:, 2:4, :], x1t_s[:, 2:4, qs], AF.Square)
        for j in range(NJ):
            nc.tensor.matmul(mu_ps, lhsT=oneC, rhs=x1t_s[:, j, qs],
                             start=(j == 0), stop=(j == NJ - 1))
        for j in range(NJ):
            nc.tensor.matmul(ssq_ps, lhsT=oneC, rhs=sq[:, j, :],
                             start=(j == 0), stop=(j == NJ - 1))

        nmu = rowpool.tile([1, 512], BF16, tag="nmu")
        nc.vector.tensor_scalar_mul(nmu, mu_ps, -1.0)
        musq = rowpool.tile([1, 512], FP32, tag="musq")
        nc.vector.tensor_mul(musq, nmu, nmu)
        varrow = rowpool.tile([1, 512], FP32, tag="var")
        nc.vector.tensor_sub(varrow, ssq_ps, musq)
        stdrow = rowpool.tile([1, 512], FP32, tag="stdr")
        nc.scalar.activation(stdrow, varrow, AF.Sqrt, bias=eps_row, scale=1.0)
        rstd1row = rowpool.tile([1, 512], FP32, tag="rs1")
        nc.vector.reciprocal(rstd1row, stdrow)
        bc1 = bcpool.tile([128, 512], FP32, tag="bc1")
        nc.gpsimd.partition_broadcast(bc1, rstd1row)

        # projection: p1u^T = P^T x1t + colsum^T (x) (-mu)
        pt = ps_mm.tile([128, 512], FP32, tag="mm")
        for j in range(NJ):
            nc.tensor.matmul(pt, lhsT=pj_s[:, j, :], rhs=x1t_s[:, j, qs],
                             start=(j == 0), stop=False)
        nc.tensor.matmul(pt, lhsT=colsum_s, rhs=nmu, start=False, stop=True)

        # p1^T = rstd1-row * p1u^T  (+ pbias column)
        if with_pbias:
            tmp = bcpool.tile([128, 512], FP32, tag="ptmp")
            nc.vector.tensor_mul(tmp, pt, bc1)
            nc.vector.tensor_scalar_add(p1T_s[:, qs], tmp, pbc_s)
        else:
            nc.vector.tensor_mul(p1T_s[:, qs], pt, bc1)

        # p1 natural (PE transpose) and p1s = rstd2 * p1
        tp = ps_tp.tile([128, 512], BF16, tag="tp")
        for t in range(4):
            w_t = 4 * q + t
            nc.tensor.transpose(tp[:, t * 128:(t + 1) * 128],
                                p1T_s[:, w_t * 128:(w_t + 1) * 128], ident)
        nc.scalar.copy(out=p1n_s[:, 4 * q:4 * (q + 1), :], in_=tp)
        for t in range(4):
            tt = 4 * q + t
            nc.vector.tensor_scalar_mul(p1s_s[:, tt, :], p1n_s[:, tt, :],
                                        rstd2_s[:, tt:tt + 1])

        # F^T accumulation for this quad's tiles
        for t in range(4):
            tt = 4 * q + t
            nc.tensor.matmul(ftp, lhsT=p1s_s[:, tt, :], rhs=x2r_s[:, tt, :],
                             start=(tt == 0), stop=False)

    # ========================================================================
    # Gram tail
    # ========================================================================
    # F^T = p1s^T @ x2 - v (x) 1,   v = (mu2*rstd2)^T @ p1n
    vtile = ps_st.tile([33, 512], FP32, tag="st")
    vps = vtile[0:1, :K2]
    for t in range(NT):
        nc.tensor.matmul(vps, lhsT=m2r2_s[:, t:t + 1], rhs=p1n_s[:, t, :],
                         start=(t == 0), stop=(t == NT - 1))
    nc.scalar.copy(out=vrow_s, in_=vps)

    nc.tensor.matmul(ftp, lhsT=vrow_s, rhs=neg_ones, start=False, stop=True)
    nc.scalar.copy(out=ft_s, in_=ftp)

    # F natural
    ftp2 = ps_tp.tile([128, 512], BF16, tag="tp")
    for j in range(NJ):
        nc.tensor.transpose(ftp2[:, j * 128:(j + 1) * 128],
                            ft_s[:, j * 128:(j + 1) * 128], ident)
    nc.scalar.copy(out=f_s, in_=ftp2)

    if with_pbias:
        sptile = ps_st.tile([33, 512], FP32, tag="st")
        sp = sptile[0:1, :K2]
        for t in range(NT):
            nc.tensor.matmul(sp, lhsT=ones_col, rhs=p1n_s[:, t, :],
                             start=(t == 0), stop=(t == NT - 1))
        nc.scalar.copy(out=s1_s, in_=sp)

    # G_h = P_h^T @ F_h (+ pb_h (x) s1_h)
    gp = ps_mm.tile([128, 512], FP32, tag="mm")
    gv = gp[:K, :HPC * K]
    for h in range(HPC):
        hs = slice(h * K, (h + 1) * K)
        for j in range(NJ):
            nc.tensor.matmul(gv[:, hs], lhsT=pj_s[:, j, hs], rhs=f_s[:, j, hs],
                             start=(j == 0),
                             stop=(j == NJ - 1) and not with_pbias)
        if with_pbias:
            nc.tensor.matmul(gv[:, hs], lhsT=pbr_s[:, hs], rhs=s1_s[:, hs],
                             start=False, stop=True)
    nc.scalar.copy(out=g_s, in_=gv)

    # H_h = M_h @ G_h; assemble block-diagonal (hbd zeroed at start)
    hp = ps_mm.tile([128, 512], FP32, tag="mm")
    hv = hp[:K, :HPC * K]
    for h in range(HPC):
        hs = slice(h * K, (h + 1) * K)
        nc.tensor.matmul(hv[:, hs], lhsT=mm_s[:, h, :], rhs=g_s[:, hs])
    for h in range(HPC):
        hs = slice(h * K, (h + 1) * K)
        nc.vector.tensor_copy(out=hbd_s[hs, hs], in_=hv[:K, hs])

    # nudged^T = H_bd^T @ p1^T;  out = nudged @ wmixT
    for q in range(NQ):
        qs = slice(q * 512, (q + 1) * 512)
        npp = ps_mm.tile([128, 512], FP32, tag="mm")
        nc.tensor.matmul(npp, lhsT=hbd_s, rhs=p1T_s[:, qs])
        nc.scalar.copy(out=nudgT_s[:, qs], in_=npp)

        stage = outpool.tile([128, 4, C], BF16, tag="ostage")
        for t in range(4):
            w_t = q * 4 + t
            mo = ps_mo.tile([128, 512], FP32, tag="mo")
            nc.tensor.matmul(mo, lhsT=nudgT_s[:, w_t * 128:(w_t + 1) * 128],
                             rhs=wmixT_s)
            if t % 2 == 0:
                nc.vector.tensor_copy(out=stage[:, t, :], in_=mo)
            else:
                nc.scalar.copy(out=stage[:, t, :], in_=mo)
        eng = nc.sync if q % 2 == 0 else nc.scalar
        eng.dma_start(
            out=outd[qs, :].rearrange("(t p) c -> p t c", p=128),
            in_=stage)


_PROGRAM_CACHE = {}


def _get_program(with_pbias: bool):
    key = ("v2", with_pbias)
    if key in _PROGRAM_CACHE:
        return _PROGRAM_CACHE[key]
    nc = bacc.Bacc("TRN2", debug=False, num_devices=NCORES)
    x1td = nc.dram_tensor("x1t", [C, W], BF16, kind="ExternalInput").ap()
    x2d = nc.dram_tensor("x2", [W, C], BF16, kind="ExternalInput").ap()
    pjd = nc.dram_tensor("pj", [C, K2], BF16, kind="ExternalInput").ap()
    colsumd = nc.dram_tensor("colsum", [1, K2], BF16, kind="ExternalInput").ap()
    mmd = nc.dram_tensor("mm", [K, HPC, K], BF16, kind="ExternalInput").ap()
    wmixTd = nc.dram_tensor("wmixT", [K2, C], BF16, kind="ExternalInput").ap()
    pbrd = pbcd = None
    if with_pbias:
        pbrd = nc.dram_tensor("pbr", [1, K2], BF16, kind="ExternalInput").ap()
        pbcd = nc.dram_tensor("pbc", [K2, 1], FP32, kind="ExternalInput").ap()
    outd = nc.dram_tensor("out", [W, C], BF16, kind="ExternalOutput").ap()
    with tile.TileContext(nc) as tc:
        with ExitStack() as ctx:
            _body(ctx, tc, x1td, x2d, pjd, colsumd, mmd, wmixTd, outd,
                  pbrd, pbcd)
    nc.compile()
    _PROGRAM_CACHE[key] = nc
    return nc


def _host_prep(inputs):
    x1 = np.asarray(inputs["x1"], np.float32)
    x2 = np.asarray(inputs["x2"], np.float32)
    gamma = np.asarray(inputs["gamma"], np.float32)
    beta = np.asarray(inputs["beta"], np.float32)
    proj = np.asarray(inputs["proj_nck"], np.float32)
    halves = np.asarray(inputs["halves"], np.float32)
    diagonals = np.asarray(inputs["diagonals"], np.float32)
    wmix = np.asarray(inputs["W_mixer"], np.float32)

    iu0, iu1 = np.triu_indices(K, k=1)
    m = np.zeros((N, K, K), np.float32)
    m[:, iu0, iu1] = halves
    m = m + np.swapaxes(m, -1, -2)
    d = np.arange(K)
    m[:, d, d] = diagonals

    pgam = proj * gamma[None, :, None]
    with_pbias = bool(np.any(beta))
    pbias = np.einsum("c,nck->nk", beta, pgam) if with_pbias else None

    x1t = [np.ascontiguousarray(x1[b].T.astype(BF)) for b in range(B)]
    x2b = [np.ascontiguousarray(x2[b].astype(BF)) for b in range(B)]

    in_maps = []
    for core in range(NCORES):
        b, hg = divmod(core, NCORES // B)
        h0 = HPC * hg
        pj_core = np.concatenate([pgam[h0 + i] for i in range(HPC)], axis=1)
        im = {
            "x1t": x1t[b],
            "x2": x2b[b],
            "pj": np.ascontiguousarray(pj_core.astype(BF)),
            "colsum": np.ascontiguousarray(
                pj_core.sum(axis=0)[None, :].astype(BF)),
            "mm": np.ascontiguousarray(
                np.stack([m[h0 + i] for i in range(HPC)], axis=1).astype(BF)),
            "wmixT": np.ascontiguousarray(
                wmix[:, K2 * hg:K2 * (hg + 1)].T.astype(BF)),
        }
        if with_pbias:
            pb = np.concatenate([pbias[h0 + i] for i in range(HPC)])
            im["pbr"] = np.ascontiguousarray(pb[None, :].astype(BF))
            im["pbc"] = np.ascontiguousarray(pb[:, None].astype(np.float32))
        in_maps.append(im)
    return in_maps, with_pbias


_TRACE = False
LAST_RESULT = None


def kernel(**inputs) -> np.ndarray:
    global LAST_RESULT
    in_maps, with_pbias = _host_prep(inputs)
    nc = _get_program(with_pbias)
    res = run_bass_kernel_spmd(nc, in_maps, core_ids=list(range(NCORES)),
                               trace=_TRACE)
    LAST_RESULT = res
    out = np.zeros((B, W, C), np.float32)
    for core in range(NCORES):
        b = core // (NCORES // B)
        out[b] += res.results[core]["out"].astype(np.float32)
    out += np.asarray(inputs["b_mixer"], np.float32)[None, None, :]
    return out


# revision 11
# speedup vs baseline: 1.3607x; 1.0268x over previous
"""Trainium2 Bass kernel for nn_MetricSelfAttention (v2, bf16).

Math: the reference's softmax is dead code, so
    nudged = (p1 @ M @ p2^T) @ p1
reassociates to
    nudged = p1 @ (M @ (p2^T @ p1))        (per-head 64x64 Gram matrix G)
collapsing the O(W^2) attention entirely.  The kernel is memory-bound.

Sharding: 8 cores = 2 batches x 4 head-pairs.  Core (b, hg) computes heads
{2hg, 2hg+1} of batch b and writes the partial mixer product
    out_partial = nudged[:, 128hg:+128] @ W_mixer[:, 128hg:+128].T
as bf16; the host sums the 4 partials per batch in fp32 and adds b_mixer.

All bulk tensors are bf16 (host-cast): halves HBM traffic vs fp32 and runs
the PE at 1 cycle/row irrespective of moving-dim size.  Per core the HBM
traffic is x1^T (2 MiB) + x2 (2 MiB) + out (2 MiB) + params (~0.3 MiB).

LayerNorm is computed on device and folded so no normalized tensor is ever
materialized:
  - x1 stats come from x1^T via PE ones-matmuls (mean and mean-square rows);
    the mean enters the projection as a rank-1 matmul (colsum (x) -mu), and
    1/std multiplies p1^T columns via one partition-broadcast row per quad
    (so the final output needs no row scaling at all: D1 rides inside p1T).
  - x2 is never normalized: with x2n = rstd2*(x2 - mu2) rowwise,
      F := x2n^T @ p1  ==  x2^T @ (rstd2*p1) - 1 (x) ((mu2*rstd2)^T @ p1)
    so F comes from raw x2 plus a rank-1 correction (v-row).
  - gamma folds into the projection on the host; nonzero beta enters as
    rank-1 bias matmuls (compiled only when beta != 0).

DMA: big transfers ride the two HWDGE queues (sync: x1t + stores,
scalar: x2 + stores), 512 KiB apiece; params go on gpsimd SWDGE.
"""

from contextlib import ExitStack

import numpy as np
import ml_dtypes

import concourse.bacc as bacc
import concourse.bass as bass
import concourse.tile as tile
from concourse import mybir
from concourse.bass_utils import run_bass_kernel_spmd
from concourse.masks import make_identity

B, W, C, N, K = 2, 2048, 512, 8, 64
NCORES = 8
HPC = 2          # heads per core
K2 = HPC * K     # 128 channels per core
EPS = 1e-5
FP32 = mybir.dt.float32
BF16 = mybir.dt.bfloat16
AF = mybir.ActivationFunctionType
OP = mybir.AluOpType
BF = ml_dtypes.bfloat16

NT = W // 128    # 16 w-tiles
NQ = W // 512    # 4 w-quads
NJ = C // 128    # 4 c-chunks


def _body(ctx: ExitStack, tc: tile.TileContext, x1td, x2d, pjd, colsumd, mmd,
          wmixTd, outd, pbrd, pbcd):
    nc = tc.nc
    with_pbias = pbrd is not None

    persist = ctx.enter_context(tc.tile_pool(name="persist", bufs=1))
    sqpool = ctx.enter_context(tc.tile_pool(name="sq", bufs=2))
    rowpool = ctx.enter_context(tc.tile_pool(name="rows", bufs=2))
    bcpool = ctx.enter_context(tc.tile_pool(name="bc", bufs=2))
    spool = ctx.enter_context(tc.tile_pool(name="stats", bufs=8))
    outpool = ctx.enter_context(tc.tile_pool(name="outstage", bufs=2))
    ps_st = ctx.enter_context(tc.tile_pool(name="ps_st", bufs=2, space="PSUM"))
    ps_mm = ctx.enter_context(tc.tile_pool(name="ps_mm", bufs=2, space="PSUM"))
    ps_tp = ctx.enter_context(tc.tile_pool(name="ps_tp", bufs=1, space="PSUM"))
    ps_f = ctx.enter_context(tc.tile_pool(name="ps_f", bufs=1, space="PSUM"))
    ps_mo = ctx.enter_context(tc.tile_pool(name="ps_mo", bufs=2, space="PSUM"))

    # ---- params (sync HWDGE; must land before the first matmul) ------------
    pj_s = persist.tile([128, NJ, K2], BF16)
    nc.sync.dma_start(out=pj_s, in_=pjd.rearrange("(j p) k -> p j k", p=128))
    colsum_s = persist.tile([1, K2], BF16)
    nc.sync.dma_start(out=colsum_s, in_=colsumd)
    wmixT_s = persist.tile([K2, C], BF16)
    nc.scalar.dma_start(out=wmixT_s, in_=wmixTd)
    mm_s = persist.tile([K, HPC, K], BF16)
    nc.scalar.dma_start(out=mm_s, in_=mmd)
    pbr_s = pbc_s = None
    if with_pbias:
        pbr_s = persist.tile([1, K2], BF16)
        nc.sync.dma_start(out=pbr_s, in_=pbrd)
        pbc_s = persist.tile([K2, 1], FP32)
        nc.sync.dma_start(out=pbc_s, in_=pbcd)
        ones_col = persist.tile([128, 1], BF16)
        nc.vector.memset(ones_col, 1.0)

    # ---- constants ---------------------------------------------------------
    neg_ones = persist.tile([1, 512], BF16)
    nc.vector.memset(neg_ones, -1.0)
    oneC = persist.tile([128, 1], BF16)
    nc.vector.memset(oneC, 1.0 / C)
    eps_s = persist.tile([128, 1], FP32)
    nc.vector.memset(eps_s, EPS)
    eps_row = persist.tile([1, 1], FP32)
    nc.vector.memset(eps_row, EPS)
    ident = persist.tile([128, 128], BF16)
    make_identity(nc, ident)

    # ---- persistent activations --------------------------------------------
    x1t_s = persist.tile([128, NJ, W], BF16)    # x1^T
    x2r_s = persist.tile([128, NT, C], BF16)    # raw x2
    p1T_s = persist.tile([K2, W], BF16)         # (D1 p1u)^T
    p1s_s = persist.tile([128, NT, K2], BF16)   # rstd2 * p1
    rstd2_s = persist.tile([128, NT], FP32)
    mu2_s = persist.tile([128, NT], BF16)       # mu2 (bf16 lhsT for v)
    ft_s = persist.tile([K2, C], BF16)          # F^T
    f_s = persist.tile([128, NJ, K2], BF16)     # F natural
    g_s = persist.tile([K, HPC * K], BF16)      # Gram (2 heads side by side)
    hbd_s = persist.tile([K2, K2], BF16)        # block-diag H
    nudgT_s = persist.tile([K2, W], BF16)       # nudged^T
    vrow_s = persist.tile([1, K2], BF16)
    if with_pbias:
        s1_s = persist.tile([1, K2], BF16)
        std2_s = persist.tile([128, NT], BF16)

    nc.vector.memset(hbd_s, 0.0)

    # ========================================================================
    # Front end, pipelined per w-quad
    # ========================================================================
    ftp = ps_f.tile([K2, C], FP32, tag="f")
    for q in range(NQ):
        qs = slice(q * 512, (q + 1) * 512)

        if q == 0:
            nc.sync.dma_start(
                out=x1t_s[:, 0:2, qs],
                in_=x1td[0:256, qs].rearrange("(j p) w -> p j w", p=128))
            nc.sync.dma_start(
                out=x1t_s[:, 2:4, qs],
                in_=x1td[256:512, qs].rearrange("(j p) w -> p j w", p=128))
        else:
            nc.sync.dma_start(
                out=x1t_s[:, :, qs],
                in_=x1td[:, qs].rearrange("(j p) w -> p j w", p=128))
        nc.scalar.dma_start(
            out=x2r_s[:, 4 * q:4 * (q + 1), :],
            in_=x2d[qs, :].rearrange("(t p) c -> p t c", p=128))

        # x2 row stats: one bn_stats per quad, batched tail math
        st6 = spool.tile([128, 4, 6], FP32, tag="bst")
        mvq = spool.tile([128, 4, 2], FP32, tag="mv")
        for t in range(4):
            nc.vector.bn_stats(st6[:, t, :], x2r_s[:, 4 * q + t, :])
            nc.vector.bn_aggr(mvq[:, t, :], st6[:, t, :])
        stdq = spool.tile([128, 4], FP32, tag="std")
        nc.scalar.activation(stdq, mvq[:, :, 1], AF.Sqrt, bias=eps_s, scale=1.0)
        nc.vector.reciprocal(rstd2_s[:, 4 * q:4 * (q + 1)], stdq)
        nc.vector.tensor_copy(out=mu2_s[:, 4 * q:4 * (q + 1)], in_=mvq[:, :, 0])
        if with_pbias:
            nc.vector.tensor_copy(out=std2_s[:, 4 * q:4 * (q + 1)], in_=stdq)

        # x1 stats rows: mu = 1/C ones^T x1t ; ssq = 1/C ones^T (x1t^2)
        st_ps = ps_st.tile([33, 512], FP32, tag="st")
        mu_ps = st_ps[0:1, :]
        ssq_ps = st_ps[32:33, :]
        sq = sqpool.tile([128, NJ, 512], BF16, tag="sq")
        nc.vector.tensor_mul(sq[:, 0:2, :], x1t_s[:, 0:2, qs], x1t_s[:, 0:2, qs])
        nc.scalar.activation(sq[:, 2:4, :], x1t_s[:, 2:4, qs], AF.Square)

        for j in range(NJ):
            nc.tensor.matmul(mu_ps, lhsT=oneC, rhs=x1t_s[:, j, qs],
                             start=(j == 0), stop=(j == NJ - 1))
        for j in range(NJ):
            nc.tensor.matmul(ssq_ps, lhsT=oneC, rhs=sq[:, j, :],
                             start=(j == 0), stop=(j == NJ - 1))

        nmu = rowpool.tile([1, 512], BF16, tag="nmu")
        nc.vector.tensor_scalar_mul(nmu, mu_ps, -1.0)
        musq = rowpool.tile([1, 512], FP32, tag="musq")
        nc.scalar.activation(musq, nmu, AF.Square)
        varrow = rowpool.tile([1, 512], FP32, tag="var")
        nc.vector.tensor_sub(varrow, ssq_ps, musq)
        stdrow = rowpool.tile([1, 512], FP32, tag="stdr")
        nc.scalar.activation(stdrow, varrow, AF.Sqrt, bias=eps_row, scale=1.0)
        rstd1row = rowpool.tile([1, 512], FP32, tag="rs1")
        nc.vector.reciprocal(rstd1row, stdrow)
        bc1 = bcpool.tile([128, 512], FP32, tag="bc1")
        nc.gpsimd.partition_broadcast(bc1, rstd1row)

        # projection: p1u^T = P^T x1t + colsum^T (x) (-mu)
        pt = ps_mm.tile([128, 512], FP32, tag="mm")
        for j in range(NJ):
            nc.tensor.matmul(pt, lhsT=pj_s[:, j, :], rhs=x1t_s[:, j, qs],
                             start=(j == 0), stop=False)
        nc.tensor.matmul(pt, lhsT=colsum_s, rhs=nmu, start=False, stop=True)

        # p1^T = rstd1-row * p1u^T  (+ pbias column)
        if with_pbias:
            tmp = bcpool.tile([128, 512], FP32, tag="ptmp")
            nc.vector.tensor_mul(tmp, pt, bc1)
            nc.vector.tensor_scalar_add(p1T_s[:, qs], tmp, pbc_s)
        else:
            nc.vector.tensor_mul(p1T_s[:, qs], pt, bc1)

        # p1 natural (PE transpose), scaled by rstd2 on the PSUM->SBUF copy
        tp = ps_tp.tile([128, 512], BF16, tag="tp")
        for t in range(4):
            w_t = 4 * q + t
            nc.tensor.transpose(tp[:, t * 128:(t + 1) * 128],
                                p1T_s[:, w_t * 128:(w_t + 1) * 128], ident)
        for t in range(4):
            tt = 4 * q + t
            nc.scalar.activation(p1s_s[:, tt, :], tp[:, t * 128:(t + 1) * 128],
                                 AF.Copy, bias=0.0,
                                 scale=rstd2_s[:, tt:tt + 1])

        # F^T accumulation for this quad's tiles
        for t in range(4):
            tt = 4 * q + t
            nc.tensor.matmul(ftp, lhsT=p1s_s[:, tt, :], rhs=x2r_s[:, tt, :],
                             start=(tt == 0), stop=False)

    # ========================================================================
    # Gram tail
    # ========================================================================
    # F^T = p1s^T @ x2 - v (x) 1,   v = (mu2*rstd2)^T @ p1n
    vtile = ps_st.tile([33, 512], FP32, tag="st")
    vps = vtile[0:1, :K2]
    for t in range(NT):
        nc.tensor.matmul(vps, lhsT=mu2_s[:, t:t + 1], rhs=p1s_s[:, t, :],
                         start=(t == 0), stop=(t == NT - 1))
    nc.scalar.copy(out=vrow_s, in_=vps)

    nc.tensor.matmul(ftp, lhsT=vrow_s, rhs=neg_ones, start=False, stop=True)
    nc.scalar.copy(out=ft_s, in_=ftp)

    # F natural
    ftp2 = ps_tp.tile([128, 512], BF16, tag="tp")
    for j in range(NJ):
        nc.tensor.transpose(ftp2[:, j * 128:(j + 1) * 128],
                            ft_s[:, j * 128:(j + 1) * 128], ident)
    nc.scalar.copy(out=f_s, in_=ftp2)

    if with_pbias:
        sptile = ps_st.tile([33, 512], FP32, tag="st")
        sp = sptile[0:1, :K2]
        for t in range(NT):
            nc.tensor.matmul(sp, lhsT=std2_s[:, t:t + 1], rhs=p1s_s[:, t, :],
                             start=(t == 0), stop=(t == NT - 1))
        nc.scalar.copy(out=s1_s, in_=sp)

    # G_h = P_h^T @ F_h (+ pb_h (x) s1_h)
    gp = ps_mm.tile([128, 512], FP32, tag="mm")
    gv = gp[:K, :HPC * K]
    for h in range(HPC):
        hs = slice(h * K, (h + 1) * K)
        for j in range(NJ):
            nc.tensor.matmul(gv[:, hs], lhsT=pj_s[:, j, hs], rhs=f_s[:, j, hs],
                             start=(j == 0),
                             stop=(j == NJ - 1) and not with_pbias)
        if with_pbias:
            nc.tensor.matmul(gv[:, hs], lhsT=pbr_s[:, hs], rhs=s1_s[:, hs],
                             start=False, stop=True)
    nc.scalar.copy(out=g_s, in_=gv)

    # H_h = M_h @ G_h; assemble block-diagonal (hbd zeroed at start)
    hp = ps_mm.tile([128, 512], FP32, tag="mm")
    hv = hp[:K, :HPC * K]
    for h in range(HPC):
        hs = slice(h * K, (h + 1) * K)
        nc.tensor.matmul(hv[:, hs], lhsT=mm_s[:, h, :], rhs=g_s[:, hs])
    for h in range(HPC):
        hs = slice(h * K, (h + 1) * K)
        nc.vector.tensor_copy(out=hbd_s[hs, hs], in_=hv[:K, hs])

    # nudged^T = H_bd^T @ p1^T;  outT_j = wmixT_j^T @ nudged^T
    for q in range(NQ):
        qs = slice(q * 512, (q + 1) * 512)
        npp = ps_mm.tile([128, 512], FP32, tag="mm")
        nc.tensor.matmul(npp, lhsT=hbd_s, rhs=p1T_s[:, qs])
        nc.scalar.copy(out=nudgT_s[:, qs], in_=npp)

    for j in range(NJ):
        stage = outpool.tile([128, NQ, 512], BF16, tag="ostage")
        for q in range(NQ):
            qs = slice(q * 512, (q + 1) * 512)
            mo = ps_mo.tile([128, 512], FP32, tag="mo")
            nc.tensor.matmul(mo, lhsT=wmixT_s[:, j * 128:(j + 1) * 128],
                             rhs=nudgT_s[:, qs])
            if q % 2 == 0:
                nc.vector.tensor_copy(out=stage[:, q, :], in_=mo)
            else:
                nc.scalar.copy(out=stage[:, q, :], in_=mo)
        eng = nc.sync if j % 2 == 0 else nc.scalar
        eng.dma_start(out=outd[j * 128:(j + 1) * 128, :], in_=stage)


_PROGRAM_CACHE = {}


def _get_program(with_pbias: bool):
    key = ("v2", with_pbias)
    if key in _PROGRAM_CACHE:
        return _PROGRAM_CACHE[key]
    nc = bacc.Bacc("TRN2", debug=False, num_devices=NCORES)
    x1td = nc.dram_tensor("x1t", [C, W], BF16, kind="ExternalInput").ap()
    x2d = nc.dram_tensor("x2", [W, C], BF16, kind="ExternalInput").ap()
    pjd = nc.dram_tensor("pj", [C, K2], BF16, kind="ExternalInput").ap()
    colsumd = nc.dram_tensor("colsum", [1, K2], BF16, kind="ExternalInput").ap()
    mmd = nc.dram_tensor("mm", [K, HPC, K], BF16, kind="ExternalInput").ap()
    wmixTd = nc.dram_tensor("wmixT", [K2, C], BF16, kind="ExternalInput").ap()
    pbrd = pbcd = None
    if with_pbias:
        pbrd = nc.dram_tensor("pbr", [1, K2], BF16, kind="ExternalInput").ap()
        pbcd = nc.dram_tensor("pbc", [K2, 1], FP32, kind="ExternalInput").ap()
    outd = nc.dram_tensor("out", [C, W], BF16, kind="ExternalOutput").ap()
    with tile.TileContext(nc) as tc:
        with ExitStack() as ctx:
            _body(ctx, tc, x1td, x2d, pjd, colsumd, mmd, wmixTd, outd,
                  pbrd, pbcd)
    nc.compile()
    _PROGRAM_CACHE[key] = nc
    return nc


def _host_prep(inputs):
    x1 = np.asarray(inputs["x1"], np.float32)
    x2 = np.asarray(inputs["x2"], np.float32)
    gamma = np.asarray(inputs["gamma"], np.float32)
    beta = np.asarray(inputs["beta"], np.float32)
    proj = np.asarray(inputs["proj_nck"], np.float32)
    halves = np.asarray(inputs["halves"], np.float32)
    diagonals = np.asarray(inputs["diagonals"], np.float32)
    wmix = np.asarray(inputs["W_mixer"], np.float32)

    iu0, iu1 = np.triu_indices(K, k=1)
    m = np.zeros((N, K, K), np.float32)
    m[:, iu0, iu1] = halves
    m = m + np.swapaxes(m, -1, -2)
    d = np.arange(K)
    m[:, d, d] = diagonals

    pgam = proj * gamma[None, :, None]
    with_pbias = bool(np.any(beta))
    pbias = np.einsum("c,nck->nk", beta, pgam) if with_pbias else None

    x1t = [np.ascontiguousarray(x1[b].T.astype(BF)) for b in range(B)]
    x2b = [np.ascontiguousarray(x2[b].astype(BF)) for b in range(B)]

    in_maps = []
    for core in range(NCORES):
        b, hg = divmod(core, NCORES // B)
        h0 = HPC * hg
        pj_core = np.concatenate([pgam[h0 + i] for i in range(HPC)], axis=1)
        im = {
            "x1t": x1t[b],
            "x2": x2b[b],
            "pj": np.ascontiguousarray(pj_core.astype(BF)),
            "colsum": np.ascontiguousarray(
                pj_core.sum(axis=0)[None, :].astype(BF)),
            "mm": np.ascontiguousarray(
                np.stack([m[h0 + i] for i in range(HPC)], axis=1).astype(BF)),
            "wmixT": np.ascontiguousarray(
                wmix[:, K2 * hg:K2 * (hg + 1)].T.astype(BF)),
        }
        if with_pbias:
            pb = np.concatenate([pbias[h0 + i] for i in range(HPC)])
            im["pbr"] = np.ascontiguousarray(pb[None, :].astype(BF))
            im["pbc"] = np.ascontiguousarray(pb[:, None].astype(np.float32))
        in_maps.append(im)
    return in_maps, with_pbias


_TRACE = False
LAST_RESULT = None


def kernel(**inputs) -> np.ndarray:
    global LAST_RESULT
    in_maps, with_pbias = _host_prep(inputs)
    nc = _get_program(with_pbias)
    res = run_bass_kernel_spmd(nc, in_maps, core_ids=list(range(NCORES)),
                               trace=_TRACE)
    LAST_RESULT = res
    out = np.zeros((B, W, C), np.float32)
    for core in range(NCORES):
        b = core // (NCORES // B)
        out[b] += res.results[core]["out"].astype(np.float32).T
    out += np.asarray(inputs["b_mixer"], np.float32)[None, None, :]
    return out
